# revision 52
# baseline (speedup 1.0000x reference)
"""CGC (Customized Gate Control) MoE layer on 8 Trainium2 NeuronCores.

Strategy: data-parallel over batch (B=4096 -> 8 shards of 512 rows); every
core computes all 8 expert MLPs for its shard — no collectives.

Precision/speed scheme (~1.7e-3 rel err vs the f32 reference on hw):
  - x, W1, Wg are uploaded as natural-scale fp8e4m3 (hi, res) pairs:
    a ~ hi + res with res = q8(a - hi), giving ~0.15% representation error.
  - Layer-1 / gate matmuls run as fp8 DoubleRow (2 contraction rows per
    instruction at 0.5 cycles/row): the three significant cross products
    (hi*hi, hi*res, res*hi) are computed by three DR instructions per
    k-tile pair using strided (hi,res) slices — 0.75 c/row/k-tile vs
    float32r's 1.0, with no operand duplication.
  - h, W2, b2, expert outputs and y are fp16 (L2 matmul at 1.0 c/row).
  - Per-tensor pow2 scales are chosen on the host; descale factors ride in
    as a small aux tensor and are applied via the ScalarE activation's
    per-partition scale operand, so nothing is baked into the compiled NEFF.

Per-core dataflow:
  - x arrives pre-transposed from the host as xp [128, kt, (hi,res), B] —
    no PE transposes or PSUM round-trips for inputs.
  - L1: hT[h1, b] psum group of 12 DR matmuls per m-tile; ScalarE applies
    relu + per-partition b1 bias + descale, writing fp16.
  - L2: oe[b, H2] fp16 matmuls; bias-add (host-precomputed b2 broadcast
    tile) + relu on VectorE.
  - Gates: DR logits (stationary padded to 16 cols for the DR ldweights
    stride rule), ScalarE descale+bias, PE transpose, softmax.
  - Gated combine: single-instruction MACs (scalar_tensor_tensor) on
    VectorE, interleaved per batch-tile into the L2 pipeline.
  - Output DMAs and the b2 broadcast load run on the idle Pool engine's
    DGE queue so they never head-of-line block the SP weight-load queue.
"""

import numpy as np
import ml_dtypes

import concourse.tile as tile
from concourse import bacc, mybir
from concourse.bass_utils import run_bass_kernel_spmd

N_CORES = 8
B = 4096
BL = B // N_CORES  # 512 rows per core
D = 1024
H1 = 1024
H2 = 512
DOM = 3
NES = 2
NSH = 2
E_SPEC = DOM * NES  # 6
GATE_K = NES + NSH  # 4
TOTAL_E = E_SPEC + NSH  # 8

F8 = mybir.dt.float8e4
F16 = mybir.dt.float16
F32 = mybir.dt.float32
AX = mybir.AxisListType
AF = mybir.ActivationFunctionType
ALU = mybir.AluOpType
DR = mybir.MatmulPerfMode.DoubleRow

NBT = BL // 128  # 4 batch tiles per core
NKD = D // 128   # 8 contraction tiles over D
NG = NKD // 2    # 4 DoubleRow k-tile pairs
NKH = H1 // 128  # 8 contraction tiles over H1
NMH = H1 // 128  # 8 output tiles over H1

NP8 = ml_dtypes.float8_e4m3fn
KPAD = 16  # gate stationary column padding (DR ldweights stride rule)

# aux tensor column map (f32 [128, 32]):
#   0..7   : L1 descale per expert e (broadcast down partitions)
#   8..11  : gate descale per gate g=0..3 (3=shared)
#   12..14 : bg[d] in rows 0..3
#   15     : bsg in rows 0..7
#   16..23 : L2 descale per expert e (1/s2[e])
AUX_DSC1 = 0
AUX_DSCG = 8
AUX_BG = 12
AUX_BSG = 15
AUX_DSC2 = 16
AUX_W = 32


def _build_nc():
    from contextlib import ExitStack

    nc = bacc.Bacc("TRN2", target_bir_lowering=False, debug=False)

    xps = [
        nc.dram_tensor(f"xp{i}", [128, NKD, 2, BL], F8, kind="ExternalInput")
        for i in range(4)
    ]
    W1p = nc.dram_tensor("W1p", [TOTAL_E, 128, NMH, NKD, 2, 128], F8,
                         kind="ExternalInput")
    W2p = nc.dram_tensor("W2p", [TOTAL_E, 128, NKH, 2, H2], F8, kind="ExternalInput")
    b1p = nc.dram_tensor("b1p", [128, TOTAL_E, NMH], F32, kind="ExternalInput")
    b28 = nc.dram_tensor("b28", [1, TOTAL_E, 2, H2], F8, kind="ExternalInput")
    ones8 = nc.dram_tensor("ones8", [1, 2, 128], F8, kind="ExternalInput")
    Wgp = nc.dram_tensor("Wgp", [DOM, 128, NKD, 2, KPAD], F8, kind="ExternalInput")
    Wsgp = nc.dram_tensor("Wsgp", [128, NKD, 2, KPAD], F8, kind="ExternalInput")
    aux = nc.dram_tensor("aux", [128, AUX_W], F32, kind="ExternalInput")
    ys = [
        nc.dram_tensor(n, [BL, H2], F16, kind="ExternalOutput")
        for n in ("y0", "y1", "y2", "ysh")
    ]

    with tile.TileContext(nc) as tc, ExitStack() as ctx:
        p_const = ctx.enter_context(tc.tile_pool(name="const", bufs=1))
        p_xp = ctx.enter_context(tc.tile_pool(name="xp", bufs=2))
        p_w1 = ctx.enter_context(tc.tile_pool(name="w1", bufs=4))
        p_w2 = ctx.enter_context(tc.tile_pool(name="w2", bufs=3))
        p_h = ctx.enter_context(tc.tile_pool(name="hT", bufs=4))
        p_oe = ctx.enter_context(tc.tile_pool(name="oe", bufs=3))
        p_osh = ctx.enter_context(tc.tile_pool(name="osh", bufs=1))
        p_acc = ctx.enter_context(tc.tile_pool(name="acc", bufs=1))
        p_gw = ctx.enter_context(tc.tile_pool(name="gw", bufs=1))
        p_gt = ctx.enter_context(tc.tile_pool(name="gt", bufs=2))
        p_sm = ctx.enter_context(tc.tile_pool(name="sm", bufs=3))
        p_tmp = ctx.enter_context(tc.tile_pool(name="tmp", bufs=2))
        ps_h = ctx.enter_context(tc.tile_pool(name="psh", bufs=3, space="PSUM"))
        ps_o = ctx.enter_context(tc.tile_pool(name="pso", bufs=3, space="PSUM"))
        ps_t = ctx.enter_context(tc.tile_pool(name="pst", bufs=2, space="PSUM"))

        # Identity for gate transposes. The PE warm-ups only need readable
        # data, so they run right after the memset; the diagonal fill
        # (affine_select) lands afterwards, well before the first transpose.
        ident_sb = p_const.tile([128, 128], F32)
        nc.gpsimd.memset(ident_sb, 0.0)
        # dummy activation preloads the ACT function table off the critical path
        dumm_sb = p_const.tile([1, 1], F32)
        nc.scalar.activation(out=dumm_sb, in_=ident_sb[:1, :1], func=AF.Relu)
        for _ in range(10):
            pw = ps_t.tile([128, 128], F32, tag="pt", name="pw")
            nc.tensor.matmul(pw, lhsT=ident_sb, rhs=ident_sb, start=True, stop=True)
        nc.gpsimd.affine_select(
            out=ident_sb,
            in_=ident_sb,
            compare_op=ALU.not_equal,
            fill=1.0,
            base=0,
            pattern=[[-1, 128]],
            channel_multiplier=1,
        )
        # b2 (fp8 DR rank-1 operands) ride the idle Pool DGE queue.
        b2_sb = p_const.tile([1, TOTAL_E, 2, H2], F8)
        nc.gpsimd.dma_start(out=b2_sb, in_=b28[:])
        ones8_sb = p_const.tile([1, 2, 128], F8)
        nc.gpsimd.dma_start(out=ones8_sb, in_=ones8[:])

        def load_xp(i):
            xp = p_xp.tile([128, NKD, 2, BL], F8, tag="xp")
            nc.sync.dma_start(out=xp, in_=xps[i][:])
            return xp

        def mm3(pg, w_sb, xp, g, start, stop, wres=True):
            """Three-term DR matmuls for k-tile pair g into psum pg.

            wres=False drops the w-residual term for this k-pair group
            (used on one of four L1 groups: costs ~7e-3 of the 2e-2 error
            budget, saves 1/12 of the L1 cycles)."""
            sl = slice(2 * g, 2 * g + 2)
            nc.tensor.matmul(pg, lhsT=w_sb[:, sl, 0, :], rhs=xp[:, sl, 0, :],
                             start=start, stop=False, perf_mode=DR)
            nc.tensor.matmul(pg, lhsT=w_sb[:, sl, 0, :], rhs=xp[:, sl, 1, :],
                             start=False, stop=(stop and not wres), perf_mode=DR)
            if wres:
                nc.tensor.matmul(pg, lhsT=w_sb[:, sl, 1, :], rhs=xp[:, sl, 0, :],
                                 start=False, stop=stop, perf_mode=DR)

        def gate_logits(xp, wg_dram):
            """x @ Wg (3-term DR) -> psum [KPAD, BL]."""
            wg_sb = p_sm.tile([128, NKD, 2, KPAD], F8, tag="wg")
            nc.sync.dma_start(out=wg_sb, in_=wg_dram)
            pg = ps_t.tile([KPAD, BL], F32, tag="pt")
            for g in range(NG):
                mm3(pg, wg_sb, xp, g, start=(g == 0), stop=(g == NG - 1))
            return pg

        def gate_softmax(pg, gi, K, tag):
            """softmax(logits + bg) -> gw tile [128, NBT, K] (b on partitions)."""
            glT = p_gt.tile([K, BL], F32, tag="glT")
            if gi < DOM:
                bias_ap = aux_sb[:K, AUX_BG + gi : AUX_BG + gi + 1]
            else:
                bias_ap = aux_sb[:K, AUX_BSG : AUX_BSG + 1]
            nc.scalar.activation(
                out=glT, in_=pg[:K, :], func=AF.Identity, bias=bias_ap,
                scale=aux_sb[:K, AUX_DSCG + gi : AUX_DSCG + gi + 1],
            )
            # softmax: logits are O(1) so exp runs without max-subtraction;
            # the exp's accumulator yields the row sums; one batched
            # reciprocal serves all four batch tiles.
            gw = p_gw.tile([128, NBT, K], F32, tag=tag)
            esb = p_sm.tile([128, NBT, K], F32, tag="esb")
            ssb = p_sm.tile([128, NBT], F32, tag="ssb")
            for bt in range(NBT):
                ptg = ps_t.tile([128, K], F32, tag="pt")
                nc.tensor.transpose(
                    ptg, glT[:, bt * 128 : (bt + 1) * 128], ident_sb[:K, :K]
                )
                nc.scalar.activation(
                    out=esb[:, bt, :], in_=ptg, func=AF.Exp, scale=1.0,
                    accum_out=ssb[:, bt : bt + 1],
                )
            rsb = p_sm.tile([128, NBT], F32, tag="rsb")
            nc.vector.reciprocal(out=rsb, in_=ssb)
            for bt in range(NBT):
                nc.vector.tensor_scalar_mul(
                    gw[:, bt, :], esb[:, bt, :], rsb[:, bt : bt + 1]
                )
            return gw

        def load_w1_part(e, mt0, nmt):
            w1_sb = p_w1.tile([128, nmt, NKD, 2, 128], F8, tag=f"w1_{nmt}")
            nc.sync.dma_start(out=w1_sb, in_=W1p[e][:, mt0 : mt0 + nmt])
            return w1_sb

        def expert_l1(xp, e, parts=None):
            """relu(x@W1+b1) -> hT [128, NMH, BL] fp16.

            parts: optional pre-issued [(w1_tile, mt0, nmt), ...] covering
            all NMH m-tiles (used to pipeline the first expert's loads).
            """
            if parts is None:
                h = NMH // 2
                parts = [(None, 0, h), (None, h, h)]
            hT = p_h.tile([128, NMH, 2, BL], F8, tag="hT")
            for w1_sb, mt0, nmt in parts:
                if w1_sb is None:
                    w1_sb = load_w1_part(e, mt0, nmt)
                for mi in range(nmt):
                    mt = mt0 + mi
                    ph = ps_h.tile([128, BL], F32, tag="ph")
                    for g in range(NG):
                        mm3(ph, w1_sb[:, mi], xp, g,
                            start=(g == 0), stop=(g == NG - 1))
                    # h as a natural-scale fp8 (hi, res) pair: ScalarE makes
                    # the fp16 value and its fp8 rounding; Pool (otherwise
                    # idle) computes the residual.
                    z16 = p_tmp.tile([128, BL], F16, tag="z16", bufs=6)
                    nc.scalar.activation(
                        out=z16,
                        in_=ph,
                        func=AF.Relu,
                        bias=b1_sb[:, e, mt : mt + 1],
                        scale=aux_sb[:, AUX_DSC1 + e : AUX_DSC1 + e + 1],
                    )
                    # spread the pair production so every engine stays under
                    # the ~1.17us/m-tile L1 matmul rate: hhi on ScalarE/Pool
                    # alternating, residual on Pool/VectorE alternating
                    if mt % 2 == 0:
                        nc.scalar.copy(out=hT[:, mt, 0, :], in_=z16)
                        nc.gpsimd.tensor_sub(hT[:, mt, 1, :], z16, hT[:, mt, 0, :])
                    else:
                        nc.gpsimd.tensor_copy(out=hT[:, mt, 0, :], in_=z16)
                        nc.vector.tensor_sub(hT[:, mt, 1, :], z16, hT[:, mt, 0, :])
            return hT

        def expert_l2(hT, e, out_pool, tag, macs=(), ydmas=()):
            """relu(h@W2+b2) -> oe [128, NBT, H2] fp16.

            macs: per-bt gated-combine hooks (acc_idx, gw, col, first),
            emitted right after each batch-tile's relu so VectorE work
            pipelines with the next tile's matmuls. ydmas: (dram, acc_idx)
            output stores emitted per-bt after the final MAC.
            """
            w2_sb = p_w2.tile([128, NKH, 2, H2], F8, tag="w2")
            nc.sync.dma_start(out=w2_sb, in_=W2p[e])
            oe = out_pool.tile([128, NBT, H2], F16, tag=tag)
            for bt in range(NBT):
                bsl = slice(bt * 128, (bt + 1) * 128)
                po = ps_o.tile([128, H2], F32, tag="po")
                for g in range(NKH // 2):
                    sl = slice(2 * g, 2 * g + 2)
                    # (hhi+hres)@w2hi + hhi@w2res
                    nc.tensor.matmul(po, lhsT=hT[:, sl, 0, bsl],
                                     rhs=w2_sb[:, sl, 0, :],
                                     start=(g == 0), stop=False, perf_mode=DR)
                    nc.tensor.matmul(po, lhsT=hT[:, sl, 1, bsl],
                                     rhs=w2_sb[:, sl, 0, :],
                                     start=False, stop=False, perf_mode=DR)
                    nc.tensor.matmul(po, lhsT=hT[:, sl, 0, bsl],
                                     rhs=w2_sb[:, sl, 1, :],
                                     start=False, stop=False, perf_mode=DR)
                # b2 bias (pre-scaled by s2[e]) as a DoubleRow rank-1
                nc.tensor.matmul(po, lhsT=ones8_sb, rhs=b2_sb[:, e],
                                 start=False, stop=True, perf_mode=DR)
                nc.scalar.activation(
                    out=oe[:, bt, :], in_=po, func=AF.Relu,
                    scale=aux_sb[:, AUX_DSC2 + e : AUX_DSC2 + e + 1],
                )
                for acc_idx, gw, col, first in macs:
                    mac(acc_idx, oe, gw, col, bt, first)
                for ydram, acc_idx in ydmas:
                    # the shared-gate output drains through the idle Pool DGE
                    # queue in parallel with the domain output on SP; the very
                    # last tile takes SP's faster HWDGE path out
                    q = nc.gpsimd if (acc_idx == 3 and bt < NBT - 1) else nc.sync
                    q.dma_start(
                        out=ydram[bt * 128 : (bt + 1) * 128, :],
                        in_=accs[acc_idx][:, bt, :],
                    )
            return oe

        accs = [None] * 4

        def mac(acc_idx, oe, gw, col, bt, first):
            acc = accs[acc_idx]
            sc = gw[:, bt, col : col + 1]
            if first:
                nc.vector.tensor_scalar_mul(acc[:, bt, :], oe[:, bt, :], sc)
            else:
                nc.vector.scalar_tensor_tensor(
                    out=acc[:, bt, :],
                    in0=oe[:, bt, :],
                    scalar=sc,
                    in1=acc[:, bt, :],
                    op0=ALU.mult,
                    op1=ALU.add,
                )

        def accumulate(acc_idx, oe, gw, col, first):
            for bt in range(NBT):
                mac(acc_idx, oe, gw, col, bt, first)

        # ---- software-pipelined schedule -------------------------------
        # Expert e's L2 is emitted after expert e+1's L1, so the per-m-tile
        # z16 -> hhi -> hres chain (ScalarE/Pool/VectorE) has a whole L1
        # window to drain before its hT is consumed.
        # DMA order at start: xp (gate + L1 dependency) first, tiny
        # wg/aux/b1 next, then e6's W1 in quarters.
        xp_sh = load_xp(3)
        pg_sh = gate_logits(xp_sh, Wsgp[:])
        aux_sb = p_const.tile([128, AUX_W], F32)
        nc.sync.dma_start(out=aux_sb, in_=aux[:])
        b1_sb = p_const.tile([128, TOTAL_E, NMH], F32)
        nc.sync.dma_start(out=b1_sb, in_=b1p[:])
        q = NMH // 4
        w1_pre = [(load_w1_part(E_SPEC, i * q, q), i * q, q) for i in range(4)]
        accs[3] = p_acc.tile([128, NBT, H2], F16, tag="acc3", name="acc3")
        hT6 = expert_l1(xp_sh, E_SPEC, parts=w1_pre)
        gws = gate_softmax(pg_sh, DOM, TOTAL_E, tag="gws")
        hT7 = expert_l1(xp_sh, E_SPEC + 1)
        xp_d = load_xp(0)
        osh = [expert_l2(hT6, E_SPEC, p_osh, tag="osh0",
                         macs=[(3, gws, E_SPEC, True)])]
        pg_d = gate_logits(xp_d, Wgp[0])
        hT_pend = expert_l1(xp_d, 0)
        osh.append(expert_l2(hT7, E_SPEC + 1, p_osh, tag="osh1",
                             macs=[(3, gws, E_SPEC + 1, False)]))

        for d in range(DOM):
            e0, e1 = d * NES, d * NES + 1
            gw_d = gate_softmax(pg_d, d, GATE_K, tag=f"gw{d}")
            accs[d] = p_acc.tile(
                [128, NBT, H2], F16, tag=f"acc{d}", name=f"acc{d}"
            )
            accumulate(d, osh[0], gw_d, NES + 0, first=True)
            accumulate(d, osh[1], gw_d, NES + 1, first=False)
            hT1 = expert_l1(xp_d, e1)
            expert_l2(hT_pend, e0, p_oe, tag="oe",
                      macs=[(d, gw_d, 0, False), (3, gws, e0, False)])
            if d < DOM - 1:
                xp_d = load_xp(d + 1)
                pg_d = gate_logits(xp_d, Wgp[d + 1])
                hT_pend = expert_l1(xp_d, (d + 1) * NES)
            ydmas = [(ys[d][:], d)]
            if d == DOM - 1:
                ydmas.append((ys[3][:], 3))
            expert_l2(hT1, e1, p_oe, tag="oe",
                      macs=[(d, gw_d, 1, False), (3, gws, e1, False)],
                      ydmas=ydmas)

    nc.compile()
    return nc


_NC_CACHE = {}


def _get_nc():
    if "nc" not in _NC_CACHE:
        _NC_CACHE["nc"] = _build_nc()
    return _NC_CACHE["nc"]


def _pow2_scale(a, target=192.0):
    m = float(np.abs(a).max())
    if m == 0.0 or not np.isfinite(m):
        return 1.0
    return float(2.0 ** np.floor(np.log2(target / m)))


def _q8(a):
    return a.astype(NP8)


def _pair(a, target=192.0):
    """a*s ~ hi + res (both natural-scale fp8). Returns (hi, res, s)."""
    s = _pow2_scale(a, target)
    asc = (a * s).astype(np.float32)
    hi = _q8(asc)
    res = _q8(asc - hi.astype(np.float32))
    return hi, res, s


def _pack_xT(x, s):
    """[BL, D] f32 -> [128, NKD, 2, BL] fp8 pair layout (d on partitions)."""
    asc = (x.astype(np.float32) * s)
    hi = _q8(asc)
    res = _q8(asc - hi.astype(np.float32))
    out = np.empty((128, NKD, 2, x.shape[0]), dtype=NP8)
    for t, arr in enumerate((hi, res)):
        # arr [BL, D] -> T [D, BL] -> [NKD, 128, BL] -> [128, NKD, BL]
        out[:, :, t, :] = arr.T.reshape(NKD, 128, -1).transpose(1, 0, 2)
    return out


def _pack_w1(Wall):
    """[E, D, H1] f32 -> ([E, 128, NMH, NKD, 2, 128] fp8, scales[E])."""
    out = np.empty((TOTAL_E, 128, NMH, NKD, 2, 128), dtype=NP8)
    scales = np.empty(TOTAL_E, dtype=np.float64)
    for e in range(TOTAL_E):
        hi, res, s = _pair(Wall[e])
        scales[e] = s
        for t, arr in enumerate((hi, res)):
            # arr [D, H1] -> [NKD, 128p, NMH, 128m] -> [128p, NMH, NKD, 128m]
            out[e, :, :, :, t, :] = (
                arr.reshape(NKD, 128, NMH, 128).transpose(1, 2, 0, 3)
            )
    return out, scales


def _pack_wg(Wg, K):
    """[D, K] f32 -> ([128, NKD, 2, KPAD] fp8 zero-padded, scale)."""
    hi, res, s = _pair(Wg)
    out = np.zeros((128, NKD, 2, KPAD), dtype=NP8)
    for t, arr in enumerate((hi, res)):
        out[:, :, t, :K] = arr.reshape(NKD, 128, K).transpose(1, 0, 2)
    return out, s


def kernel(**inputs):
    return run_kernel(inputs)


def run_kernel(inputs, trace=False):
    nc = _get_nc()
    f = {k: np.ascontiguousarray(np.asarray(v, dtype=np.float32))
         for k, v in inputs.items()}

    W1all = np.concatenate([f["W1s"], f["W1h"]], axis=0)
    W2all = np.concatenate([f["W2s"], f["W2h"]], axis=0)
    b1all = np.concatenate([f["b1s"], f["b1h"]], axis=0)
    b2all = np.concatenate([f["b2s"], f["b2h"]], axis=0)

    W1p, s1 = _pack_w1(W1all)
    # W2 as fp8 (hi, res) pairs, [E, 128, NKH, 2, H2]
    W2p = np.empty((TOTAL_E, 128, NKH, 2, H2), dtype=NP8)
    s2 = np.empty(TOTAL_E, dtype=np.float64)
    for e in range(TOTAL_E):
        hi, res, s = _pair(W2all[e])
        s2[e] = s
        for t, arr in enumerate((hi, res)):
            W2p[e, :, :, t, :] = arr.reshape(NKH, 128, H2).transpose(1, 0, 2)
    # b1p[p, e, mt] = b1[e, mt*128 + p]
    b1p = np.ascontiguousarray(b1all.reshape(TOTAL_E, NMH, 128).transpose(2, 0, 1))
    # b2 rides at the per-expert W2 scale so the whole L2 psum shares one descale
    b28 = np.zeros((1, TOTAL_E, 2, H2), dtype=NP8)
    for e in range(TOTAL_E):
        b28[0, e, 0, :] = np.clip(b2all[e] * s2[e], -224, 224).astype(NP8)
    ones8 = np.zeros((1, 2, 128), dtype=NP8)
    ones8[0, 0, :] = 1.0

    xs_full = [f["x0"], f["x1"], f["x2"], f["x_shared"]]
    sx = [_pow2_scale(x) for x in xs_full]

    wg_packs = [_pack_wg(f["Wg"][d], GATE_K) for d in range(DOM)]
    Wgp = np.ascontiguousarray(np.stack([w for w, _ in wg_packs]))
    Wsgp, sgs = _pack_wg(f["Wsg"], TOTAL_E)
    Wsgp = np.ascontiguousarray(Wsgp)

    aux = np.zeros((128, AUX_W), dtype=np.float32)
    for e in range(TOTAL_E):
        xd = e // NES if e < E_SPEC else 3
        aux[:, AUX_DSC1 + e] = 1.0 / (sx[xd] * s1[e])
        aux[:, AUX_DSC2 + e] = 1.0 / s2[e]
    for d in range(DOM):
        aux[:, AUX_DSCG + d] = 1.0 / (sx[d] * wg_packs[d][1])
        aux[:GATE_K, AUX_BG + d] = f["bg"][d]
    aux[:, AUX_DSCG + 3] = 1.0 / (sx[3] * sgs)
    aux[:TOTAL_E, AUX_BSG] = f["bsg"]

    common = {
        "W1p": W1p, "W2p": W2p, "b1p": b1p, "b28": b28, "ones8": ones8,
        "Wgp": Wgp, "Wsgp": Wsgp, "aux": aux,
    }
    in_maps = []
    for c in range(N_CORES):
        m = dict(common)
        for i, name in enumerate(("x0", "x1", "x2", "x_shared")):
            shard = f[name][c * BL : (c + 1) * BL]
            m[f"xp{i}"] = _pack_xT(shard, sx[i])
        in_maps.append(m)

    res = run_bass_kernel_spmd(nc, in_maps, list(range(N_CORES)), trace=trace)
    outs = []
    for name in ("y0", "y1", "y2", "ysh"):
        outs.append(
            np.concatenate(
                [np.asarray(res.results[c][name]).astype(np.float32)
                 for c in range(N_CORES)],
                axis=0,
            )
        )
    out = tuple(outs)
    if trace:
        return out, res
    return out


# revision 54
# speedup vs baseline: 1.0053x; 1.0053x over previous
"""CGC (Customized Gate Control) MoE layer on 8 Trainium2 NeuronCores.

Strategy: data-parallel over batch (B=4096 -> 8 shards of 512 rows); every
core computes all 8 expert MLPs for its shard — no collectives.

Precision/speed scheme (~1.7e-3 rel err vs the f32 reference on hw):
  - x, W1, Wg are uploaded as natural-scale fp8e4m3 (hi, res) pairs:
    a ~ hi + res with res = q8(a - hi), giving ~0.15% representation error.
  - Layer-1 / gate matmuls run as fp8 DoubleRow (2 contraction rows per
    instruction at 0.5 cycles/row): the three significant cross products
    (hi*hi, hi*res, res*hi) are computed by three DR instructions per
    k-tile pair using strided (hi,res) slices — 0.75 c/row/k-tile vs
    float32r's 1.0, with no operand duplication.
  - h, W2, b2, expert outputs and y are fp16 (L2 matmul at 1.0 c/row).
  - Per-tensor pow2 scales are chosen on the host; descale factors ride in
    as a small aux tensor and are applied via the ScalarE activation's
    per-partition scale operand, so nothing is baked into the compiled NEFF.

Per-core dataflow:
  - x arrives pre-transposed from the host as xp [128, kt, (hi,res), B] —
    no PE transposes or PSUM round-trips for inputs.
  - L1: hT[h1, b] psum group of 12 DR matmuls per m-tile; ScalarE applies
    relu + per-partition b1 bias + descale, writing fp16.
  - L2: oe[b, H2] fp16 matmuls; bias-add (host-precomputed b2 broadcast
    tile) + relu on VectorE.
  - Gates: DR logits (stationary padded to 16 cols for the DR ldweights
    stride rule), ScalarE descale+bias, PE transpose, softmax.
  - Gated combine: single-instruction MACs (scalar_tensor_tensor) on
    VectorE, interleaved per batch-tile into the L2 pipeline.
  - Output DMAs and the b2 broadcast load run on the idle Pool engine's
    DGE queue so they never head-of-line block the SP weight-load queue.
"""

import numpy as np
import ml_dtypes

import concourse.tile as tile
from concourse import bacc, mybir
from concourse.bass_utils import run_bass_kernel_spmd

N_CORES = 8
B = 4096
BL = B // N_CORES  # 512 rows per core
D = 1024
H1 = 1024
H2 = 512
DOM = 3
NES = 2
NSH = 2
E_SPEC = DOM * NES  # 6
GATE_K = NES + NSH  # 4
TOTAL_E = E_SPEC + NSH  # 8

F8 = mybir.dt.float8e4
F16 = mybir.dt.float16
F32 = mybir.dt.float32
AX = mybir.AxisListType
AF = mybir.ActivationFunctionType
ALU = mybir.AluOpType
DR = mybir.MatmulPerfMode.DoubleRow

NBT = BL // 128  # 4 batch tiles per core
NKD = D // 128   # 8 contraction tiles over D
NG = NKD // 2    # 4 DoubleRow k-tile pairs
NKH = H1 // 128  # 8 contraction tiles over H1
NMH = H1 // 128  # 8 output tiles over H1

NP8 = ml_dtypes.float8_e4m3fn
KPAD = 16  # gate stationary column padding (DR ldweights stride rule)

# aux tensor column map (f32 [128, 32]):
#   0..7   : L1 descale per expert e (broadcast down partitions)
#   8..11  : gate descale per gate g=0..3 (3=shared)
#   12..14 : bg[d] in rows 0..3
#   15     : bsg in rows 0..7
#   16..23 : L2 descale per expert e (1/s2[e])
AUX_DSC1 = 0
AUX_DSCG = 8
AUX_BG = 12
AUX_BSG = 15
AUX_DSC2 = 16
AUX_W = 32


def _build_nc():
    from contextlib import ExitStack

    nc = bacc.Bacc("TRN2", target_bir_lowering=False, debug=False)

    xps = [
        nc.dram_tensor(f"xp{i}", [128, NKD, 2, BL], F8, kind="ExternalInput")
        for i in range(4)
    ]
    W1p = nc.dram_tensor("W1p", [TOTAL_E, 128, NMH, NKD, 2, 128], F8,
                         kind="ExternalInput")
    W2p = nc.dram_tensor("W2p", [TOTAL_E, 128, NKH, 2, H2], F8, kind="ExternalInput")
    b1p = nc.dram_tensor("b1p", [128, TOTAL_E, NMH], F32, kind="ExternalInput")
    b28 = nc.dram_tensor("b28", [1, TOTAL_E, 2, H2], F8, kind="ExternalInput")
    ones8 = nc.dram_tensor("ones8", [1, 2, 128], F8, kind="ExternalInput")
    Wgp = nc.dram_tensor("Wgp", [DOM, 128, NKD, 2, KPAD], F8, kind="ExternalInput")
    Wsgp = nc.dram_tensor("Wsgp", [128, NKD, 2, KPAD], F8, kind="ExternalInput")
    aux = nc.dram_tensor("aux", [128, AUX_W], F32, kind="ExternalInput")
    ys = [
        nc.dram_tensor(n, [BL, H2], F16, kind="ExternalOutput")
        for n in ("y0", "y1", "y2", "ysh")
    ]

    with tile.TileContext(nc) as tc, ExitStack() as ctx:
        p_const = ctx.enter_context(tc.tile_pool(name="const", bufs=1))
        p_xp = ctx.enter_context(tc.tile_pool(name="xp", bufs=2))
        p_w1 = ctx.enter_context(tc.tile_pool(name="w1", bufs=4))
        p_w2 = ctx.enter_context(tc.tile_pool(name="w2", bufs=3))
        p_h = ctx.enter_context(tc.tile_pool(name="hT", bufs=4))
        p_oe = ctx.enter_context(tc.tile_pool(name="oe", bufs=3))
        p_osh = ctx.enter_context(tc.tile_pool(name="osh", bufs=1))
        p_acc = ctx.enter_context(tc.tile_pool(name="acc", bufs=1))
        p_gw = ctx.enter_context(tc.tile_pool(name="gw", bufs=1))
        p_gt = ctx.enter_context(tc.tile_pool(name="gt", bufs=2))
        p_sm = ctx.enter_context(tc.tile_pool(name="sm", bufs=3))
        p_tmp = ctx.enter_context(tc.tile_pool(name="tmp", bufs=2))
        ps_h = ctx.enter_context(tc.tile_pool(name="psh", bufs=3, space="PSUM"))
        ps_o = ctx.enter_context(tc.tile_pool(name="pso", bufs=3, space="PSUM"))
        ps_t = ctx.enter_context(tc.tile_pool(name="pst", bufs=2, space="PSUM"))

        # Identity for gate transposes. The PE warm-ups only need readable
        # data, so they run right after the memset; the diagonal fill
        # (affine_select) lands afterwards, well before the first transpose.
        ident_sb = p_const.tile([128, 128], F32)
        nc.vector.memset(ident_sb, 0.0)
        # dummy activation preloads the ACT function table off the critical path
        dumm_sb = p_const.tile([1, 1], F32)
        nc.scalar.activation(out=dumm_sb, in_=ident_sb[:1, :1], func=AF.Relu)
        for _ in range(10):
            pw = ps_t.tile([128, 128], F32, tag="pt", name="pw")
            nc.tensor.matmul(pw, lhsT=ident_sb, rhs=ident_sb, start=True, stop=True)
        nc.gpsimd.affine_select(
            out=ident_sb,
            in_=ident_sb,
            compare_op=ALU.not_equal,
            fill=1.0,
            base=0,
            pattern=[[-1, 128]],
            channel_multiplier=1,
        )
        # b2 (fp8 DR rank-1 operands) ride the idle Pool DGE queue.
        b2_sb = p_const.tile([1, TOTAL_E, 2, H2], F8)
        nc.gpsimd.dma_start(out=b2_sb, in_=b28[:])
        ones8_sb = p_const.tile([1, 2, 128], F8)
        nc.gpsimd.dma_start(out=ones8_sb, in_=ones8[:])

        def load_xp(i):
            xp = p_xp.tile([128, NKD, 2, BL], F8, tag="xp")
            nc.sync.dma_start(out=xp, in_=xps[i][:])
            return xp

        def mm3(pg, w_sb, xp, g, start, stop, wres=True):
            """Three-term DR matmuls for k-tile pair g into psum pg.

            wres=False drops the w-residual term for this k-pair group
            (used on one of four L1 groups: costs ~7e-3 of the 2e-2 error
            budget, saves 1/12 of the L1 cycles)."""
            sl = slice(2 * g, 2 * g + 2)
            nc.tensor.matmul(pg, lhsT=w_sb[:, sl, 0, :], rhs=xp[:, sl, 0, :],
                             start=start, stop=False, perf_mode=DR)
            nc.tensor.matmul(pg, lhsT=w_sb[:, sl, 0, :], rhs=xp[:, sl, 1, :],
                             start=False, stop=(stop and not wres), perf_mode=DR)
            if wres:
                nc.tensor.matmul(pg, lhsT=w_sb[:, sl, 1, :], rhs=xp[:, sl, 0, :],
                                 start=False, stop=stop, perf_mode=DR)

        def gate_logits(xp, wg_dram):
            """x @ Wg (3-term DR) -> psum [KPAD, BL]."""
            wg_sb = p_sm.tile([128, NKD, 2, KPAD], F8, tag="wg")
            nc.sync.dma_start(out=wg_sb, in_=wg_dram)
            pg = ps_t.tile([KPAD, BL], F32, tag="pt")
            for g in range(NG):
                mm3(pg, wg_sb, xp, g, start=(g == 0), stop=(g == NG - 1))
            return pg

        def gate_softmax(pg, gi, K, tag):
            """softmax(logits + bg) -> gw tile [128, NBT, K] (b on partitions)."""
            glT = p_gt.tile([K, BL], F32, tag="glT")
            if gi < DOM:
                bias_ap = aux_sb[:K, AUX_BG + gi : AUX_BG + gi + 1]
            else:
                bias_ap = aux_sb[:K, AUX_BSG : AUX_BSG + 1]
            nc.scalar.activation(
                out=glT, in_=pg[:K, :], func=AF.Identity, bias=bias_ap,
                scale=aux_sb[:K, AUX_DSCG + gi : AUX_DSCG + gi + 1],
            )
            # softmax: logits are O(1) so exp runs without max-subtraction;
            # the exp's accumulator yields the row sums; one batched
            # reciprocal serves all four batch tiles.
            gw = p_gw.tile([128, NBT, K], F32, tag=tag)
            esb = p_sm.tile([128, NBT, K], F32, tag="esb")
            ssb = p_sm.tile([128, NBT], F32, tag="ssb")
            for bt in range(NBT):
                ptg = ps_t.tile([128, K], F32, tag="pt")
                nc.tensor.transpose(
                    ptg, glT[:, bt * 128 : (bt + 1) * 128], ident_sb[:K, :K]
                )
                nc.scalar.activation(
                    out=esb[:, bt, :], in_=ptg, func=AF.Exp, scale=1.0,
                    accum_out=ssb[:, bt : bt + 1],
                )
            rsb = p_sm.tile([128, NBT], F32, tag="rsb")
            nc.vector.reciprocal(out=rsb, in_=ssb)
            for bt in range(NBT):
                nc.vector.tensor_scalar_mul(
                    gw[:, bt, :], esb[:, bt, :], rsb[:, bt : bt + 1]
                )
            return gw

        def load_w1_part(e, mt0, nmt):
            w1_sb = p_w1.tile([128, nmt, NKD, 2, 128], F8, tag=f"w1_{nmt}")
            nc.sync.dma_start(out=w1_sb, in_=W1p[e][:, mt0 : mt0 + nmt])
            return w1_sb

        def expert_l1(xp, e, parts=None):
            """relu(x@W1+b1) -> hT [128, NMH, BL] fp16.

            parts: optional pre-issued [(w1_tile, mt0, nmt), ...] covering
            all NMH m-tiles (used to pipeline the first expert's loads).
            """
            if parts is None:
                h = NMH // 2
                parts = [(None, 0, h), (None, h, h)]
            hT = p_h.tile([128, NMH, 2, BL], F8, tag="hT")
            for w1_sb, mt0, nmt in parts:
                if w1_sb is None:
                    w1_sb = load_w1_part(e, mt0, nmt)
                for mi in range(nmt):
                    mt = mt0 + mi
                    ph = ps_h.tile([128, BL], F32, tag="ph")
                    for g in range(NG):
                        mm3(ph, w1_sb[:, mi], xp, g,
                            start=(g == 0), stop=(g == NG - 1))
                    # h as a natural-scale fp8 (hi, res) pair: ScalarE makes
                    # the fp16 value and its fp8 rounding; Pool (otherwise
                    # idle) computes the residual.
                    z16 = p_tmp.tile([128, BL], F16, tag="z16", bufs=6)
                    nc.scalar.activation(
                        out=z16,
                        in_=ph,
                        func=AF.Relu,
                        bias=b1_sb[:, e, mt : mt + 1],
                        scale=aux_sb[:, AUX_DSC1 + e : AUX_DSC1 + e + 1],
                    )
                    # spread the pair production so every engine stays under
                    # the ~1.17us/m-tile L1 matmul rate: hhi on ScalarE/Pool
                    # alternating, residual on Pool/VectorE alternating
                    if mt % 2 == 0:
                        nc.scalar.copy(out=hT[:, mt, 0, :], in_=z16)
                        nc.gpsimd.tensor_sub(hT[:, mt, 1, :], z16, hT[:, mt, 0, :])
                    else:
                        nc.gpsimd.tensor_copy(out=hT[:, mt, 0, :], in_=z16)
                        nc.vector.tensor_sub(hT[:, mt, 1, :], z16, hT[:, mt, 0, :])
            return hT

        def expert_l2(hT, e, out_pool, tag, macs=(), ydmas=()):
            """relu(h@W2+b2) -> oe [128, NBT, H2] fp16.

            macs: per-bt gated-combine hooks (acc_idx, gw, col, first),
            emitted right after each batch-tile's relu so VectorE work
            pipelines with the next tile's matmuls. ydmas: (dram, acc_idx)
            output stores emitted per-bt after the final MAC.
            """
            w2_sb = p_w2.tile([128, NKH, 2, H2], F8, tag="w2")
            nc.sync.dma_start(out=w2_sb, in_=W2p[e])
            oe = out_pool.tile([128, NBT, H2], F16, tag=tag)
            for bt in range(NBT):
                bsl = slice(bt * 128, (bt + 1) * 128)
                po = ps_o.tile([128, H2], F32, tag="po")
                for g in range(NKH // 2):
                    sl = slice(2 * g, 2 * g + 2)
                    # (hhi+hres)@w2hi + hhi@w2res
                    nc.tensor.matmul(po, lhsT=hT[:, sl, 0, bsl],
                                     rhs=w2_sb[:, sl, 0, :],
                                     start=(g == 0), stop=False, perf_mode=DR)
                    nc.tensor.matmul(po, lhsT=hT[:, sl, 1, bsl],
                                     rhs=w2_sb[:, sl, 0, :],
                                     start=False, stop=False, perf_mode=DR)
                    nc.tensor.matmul(po, lhsT=hT[:, sl, 0, bsl],
                                     rhs=w2_sb[:, sl, 1, :],
                                     start=False, stop=False, perf_mode=DR)
                # b2 bias (pre-scaled by s2[e]) as a DoubleRow rank-1
                nc.tensor.matmul(po, lhsT=ones8_sb, rhs=b2_sb[:, e],
                                 start=False, stop=True, perf_mode=DR)
                nc.scalar.activation(
                    out=oe[:, bt, :], in_=po, func=AF.Relu,
                    scale=aux_sb[:, AUX_DSC2 + e : AUX_DSC2 + e + 1],
                )
                for acc_idx, gw, col, first in macs:
                    mac(acc_idx, oe, gw, col, bt, first)
                for ydram, acc_idx in ydmas:
                    # the shared-gate output drains through the idle Pool DGE
                    # queue in parallel with the domain output on SP; the very
                    # last tile takes SP's faster HWDGE path out
                    q = nc.gpsimd if (acc_idx == 3 and bt < NBT - 1) else nc.sync
                    q.dma_start(
                        out=ydram[bt * 128 : (bt + 1) * 128, :],
                        in_=accs[acc_idx][:, bt, :],
                    )
            return oe

        accs = [None] * 4

        def mac(acc_idx, oe, gw, col, bt, first):
            acc = accs[acc_idx]
            sc = gw[:, bt, col : col + 1]
            if first:
                nc.vector.tensor_scalar_mul(acc[:, bt, :], oe[:, bt, :], sc)
            else:
                nc.vector.scalar_tensor_tensor(
                    out=acc[:, bt, :],
                    in0=oe[:, bt, :],
                    scalar=sc,
                    in1=acc[:, bt, :],
                    op0=ALU.mult,
                    op1=ALU.add,
                )

        def accumulate(acc_idx, oe, gw, col, first):
            for bt in range(NBT):
                mac(acc_idx, oe, gw, col, bt, first)

        # ---- software-pipelined schedule -------------------------------
        # Expert e's L2 is emitted after expert e+1's L1, so the per-m-tile
        # z16 -> hhi -> hres chain (ScalarE/Pool/VectorE) has a whole L1
        # window to drain before its hT is consumed.
        # DMA order at start: xp (gate + L1 dependency) first, tiny
        # wg/aux/b1 next, then e6's W1 in quarters.
        xp_sh = load_xp(3)
        pg_sh = gate_logits(xp_sh, Wsgp[:])
        aux_sb = p_const.tile([128, AUX_W], F32)
        nc.sync.dma_start(out=aux_sb, in_=aux[:])
        b1_sb = p_const.tile([128, TOTAL_E, NMH], F32)
        nc.sync.dma_start(out=b1_sb, in_=b1p[:])
        w1_pre = [(load_w1_part(E_SPEC, i, 1), i, 1) for i in range(NMH)]
        accs[3] = p_acc.tile([128, NBT, H2], F16, tag="acc3", name="acc3")
        hT6 = expert_l1(xp_sh, E_SPEC, parts=w1_pre)
        gws = gate_softmax(pg_sh, DOM, TOTAL_E, tag="gws")
        hT7 = expert_l1(xp_sh, E_SPEC + 1)
        xp_d = load_xp(0)
        osh = [expert_l2(hT6, E_SPEC, p_osh, tag="osh0",
                         macs=[(3, gws, E_SPEC, True)])]
        pg_d = gate_logits(xp_d, Wgp[0])
        hT_pend = expert_l1(xp_d, 0)
        osh.append(expert_l2(hT7, E_SPEC + 1, p_osh, tag="osh1",
                             macs=[(3, gws, E_SPEC + 1, False)]))

        for d in range(DOM):
            e0, e1 = d * NES, d * NES + 1
            gw_d = gate_softmax(pg_d, d, GATE_K, tag=f"gw{d}")
            accs[d] = p_acc.tile(
                [128, NBT, H2], F16, tag=f"acc{d}", name=f"acc{d}"
            )
            accumulate(d, osh[0], gw_d, NES + 0, first=True)
            accumulate(d, osh[1], gw_d, NES + 1, first=False)
            hT1 = expert_l1(xp_d, e1)
            expert_l2(hT_pend, e0, p_oe, tag="oe",
                      macs=[(d, gw_d, 0, False), (3, gws, e0, False)])
            if d < DOM - 1:
                xp_d = load_xp(d + 1)
                pg_d = gate_logits(xp_d, Wgp[d + 1])
                hT_pend = expert_l1(xp_d, (d + 1) * NES)
            ydmas = [(ys[d][:], d)]
            if d == DOM - 1:
                ydmas.append((ys[3][:], 3))
            expert_l2(hT1, e1, p_oe, tag="oe",
                      macs=[(d, gw_d, 1, False), (3, gws, e1, False)],
                      ydmas=ydmas)

    nc.compile()
    return nc


_NC_CACHE = {}


def _get_nc():
    if "nc" not in _NC_CACHE:
        _NC_CACHE["nc"] = _build_nc()
    return _NC_CACHE["nc"]


def _pow2_scale(a, target=192.0):
    m = float(np.abs(a).max())
    if m == 0.0 or not np.isfinite(m):
        return 1.0
    return float(2.0 ** np.floor(np.log2(target / m)))


def _q8(a):
    return a.astype(NP8)


def _pair(a, target=192.0):
    """a*s ~ hi + res (both natural-scale fp8). Returns (hi, res, s)."""
    s = _pow2_scale(a, target)
    asc = (a * s).astype(np.float32)
    hi = _q8(asc)
    res = _q8(asc - hi.astype(np.float32))
    return hi, res, s


def _pack_xT(x, s):
    """[BL, D] f32 -> [128, NKD, 2, BL] fp8 pair layout (d on partitions)."""
    asc = (x.astype(np.float32) * s)
    hi = _q8(asc)
    res = _q8(asc - hi.astype(np.float32))
    out = np.empty((128, NKD, 2, x.shape[0]), dtype=NP8)
    for t, arr in enumerate((hi, res)):
        # arr [BL, D] -> T [D, BL] -> [NKD, 128, BL] -> [128, NKD, BL]
        out[:, :, t, :] = arr.T.reshape(NKD, 128, -1).transpose(1, 0, 2)
    return out


def _pack_w1(Wall):
    """[E, D, H1] f32 -> ([E, 128, NMH, NKD, 2, 128] fp8, scales[E])."""
    out = np.empty((TOTAL_E, 128, NMH, NKD, 2, 128), dtype=NP8)
    scales = np.empty(TOTAL_E, dtype=np.float64)
    for e in range(TOTAL_E):
        hi, res, s = _pair(Wall[e])
        scales[e] = s
        for t, arr in enumerate((hi, res)):
            # arr [D, H1] -> [NKD, 128p, NMH, 128m] -> [128p, NMH, NKD, 128m]
            out[e, :, :, :, t, :] = (
                arr.reshape(NKD, 128, NMH, 128).transpose(1, 2, 0, 3)
            )
    return out, scales


def _pack_wg(Wg, K):
    """[D, K] f32 -> ([128, NKD, 2, KPAD] fp8 zero-padded, scale)."""
    hi, res, s = _pair(Wg)
    out = np.zeros((128, NKD, 2, KPAD), dtype=NP8)
    for t, arr in enumerate((hi, res)):
        out[:, :, t, :K] = arr.reshape(NKD, 128, K).transpose(1, 0, 2)
    return out, s


def kernel(**inputs):
    return run_kernel(inputs)


def run_kernel(inputs, trace=False):
    nc = _get_nc()
    f = {k: np.ascontiguousarray(np.asarray(v, dtype=np.float32))
         for k, v in inputs.items()}

    W1all = np.concatenate([f["W1s"], f["W1h"]], axis=0)
    W2all = np.concatenate([f["W2s"], f["W2h"]], axis=0)
    b1all = np.concatenate([f["b1s"], f["b1h"]], axis=0)
    b2all = np.concatenate([f["b2s"], f["b2h"]], axis=0)

    W1p, s1 = _pack_w1(W1all)
    # W2 as fp8 (hi, res) pairs, [E, 128, NKH, 2, H2]
    W2p = np.empty((TOTAL_E, 128, NKH, 2, H2), dtype=NP8)
    s2 = np.empty(TOTAL_E, dtype=np.float64)
    for e in range(TOTAL_E):
        hi, res, s = _pair(W2all[e])
        s2[e] = s
        for t, arr in enumerate((hi, res)):
            W2p[e, :, :, t, :] = arr.reshape(NKH, 128, H2).transpose(1, 0, 2)
    # b1p[p, e, mt] = b1[e, mt*128 + p]
    b1p = np.ascontiguousarray(b1all.reshape(TOTAL_E, NMH, 128).transpose(2, 0, 1))
    # b2 rides at the per-expert W2 scale so the whole L2 psum shares one descale
    b28 = np.zeros((1, TOTAL_E, 2, H2), dtype=NP8)
    for e in range(TOTAL_E):
        b28[0, e, 0, :] = np.clip(b2all[e] * s2[e], -224, 224).astype(NP8)
    ones8 = np.zeros((1, 2, 128), dtype=NP8)
    ones8[0, 0, :] = 1.0

    xs_full = [f["x0"], f["x1"], f["x2"], f["x_shared"]]
    sx = [_pow2_scale(x) for x in xs_full]

    wg_packs = [_pack_wg(f["Wg"][d], GATE_K) for d in range(DOM)]
    Wgp = np.ascontiguousarray(np.stack([w for w, _ in wg_packs]))
    Wsgp, sgs = _pack_wg(f["Wsg"], TOTAL_E)
    Wsgp = np.ascontiguousarray(Wsgp)

    aux = np.zeros((128, AUX_W), dtype=np.float32)
    for e in range(TOTAL_E):
        xd = e // NES if e < E_SPEC else 3
        aux[:, AUX_DSC1 + e] = 1.0 / (sx[xd] * s1[e])
        aux[:, AUX_DSC2 + e] = 1.0 / s2[e]
    for d in range(DOM):
        aux[:, AUX_DSCG + d] = 1.0 / (sx[d] * wg_packs[d][1])
        aux[:GATE_K, AUX_BG + d] = f["bg"][d]
    aux[:, AUX_DSCG + 3] = 1.0 / (sx[3] * sgs)
    aux[:TOTAL_E, AUX_BSG] = f["bsg"]

    common = {
        "W1p": W1p, "W2p": W2p, "b1p": b1p, "b28": b28, "ones8": ones8,
        "Wgp": Wgp, "Wsgp": Wsgp, "aux": aux,
    }
    in_maps = []
    for c in range(N_CORES):
        m = dict(common)
        for i, name in enumerate(("x0", "x1", "x2", "x_shared")):
            shard = f[name][c * BL : (c + 1) * BL]
            m[f"xp{i}"] = _pack_xT(shard, sx[i])
        in_maps.append(m)

    res = run_bass_kernel_spmd(nc, in_maps, list(range(N_CORES)), trace=trace)
    outs = []
    for name in ("y0", "y1", "y2", "ysh"):
        outs.append(
            np.concatenate(
                [np.asarray(res.results[c][name]).astype(np.float32)
                 for c in range(N_CORES)],
                axis=0,
            )
        )
    out = tuple(outs)
    if trace:
        return out, res
    return out
